# revision 1
# baseline (speedup 1.0000x reference)
"""DeepFM dense-MLP kernel for 8x Trainium2 NeuronCores (Bass/Tile).

Computation (reference):
    inter = relu(x * x.sum(axis=1, keepdims=True))        # FM pairwise term
    h = x
    for i in 0..3:  h = relu(h @ W_dnn[i].T + b_dnn[i])
    out = ((h + inter) * 0.5) @ W_out.T + b_out

Strategy:
  - Data-parallel: batch B=8192 split across 8 cores (1024 rows each).
  - Feature-major activations on device: h^T [D, B_c] so every GEMM is
    psum[e, b] += W^T[d_k, e_m].T @ h^T[d_k, b]  with the weight tile
    stationary and activations streaming (N=512 per matmul).
  - dtype config: bf16 (fast weight load, weights streamed once per
    layer, both 512-col passes share each weight strip) or float32r
    (fp32 storage at full PE rate, 2 super-passes, weights streamed
    twice).
  - PSUM evictions are single fused DVE ops: (psum + bias) max 0.
    The 0.5 scale on the last-layer input is folded into layer-4's
    weights and bias on the host.
  - Row-sum s = sum_d x[b, d] computed on PE with a ones-vector matmul;
    broadcast of 0.5*s across partitions via a K=1 matmul; the FM term
    is fused into the last-layer input build:
        h5in = 0.5*h4 + relu(x^T * 0.5 s).
"""

import sys

import ml_dtypes
import numpy as np

if "/opt/trn_rl_repo" not in sys.path:
    sys.path.insert(0, "/opt/trn_rl_repo")

import concourse.mybir as mybir  # noqa: E402
import concourse.tile as tile  # noqa: E402
from concourse import bacc  # noqa: E402
from concourse.bass_utils import run_bass_kernel_spmd  # noqa: E402

B, D, L = 8192, 4096, 4
NCORES = 8
BC = B // NCORES  # 1024 batch rows per core
P = 128
KK = D // P  # 32 k-tiles
MM = D // P  # 32 m-tiles
NB = 512  # matmul free dim / PSUM bank
NLAYERS = 5

USE_F32R = False  # False -> bfloat16 matmuls

f32 = mybir.dt.float32
f32r = mybir.dt.float32r
bf16 = mybir.dt.bfloat16

if USE_F32R:
    DT = f32r
    NPDT = np.float32
    S = 2  # super-passes (weights streamed once per super-pass)
    WBUFS = 2
else:
    DT = bf16
    NPDT = ml_dtypes.bfloat16
    S = 1
    WBUFS = 6

COLS = BC // S  # columns per super-pass
PI = COLS // NB  # inner passes per super-pass


def _build():
    nc = bacc.Bacc(None, target_bir_lowering=False, debug=False)
    xt_p = nc.declare_dram_parameter("xt", [KK, P, BC], DT, isOutput=False)
    w_p = nc.declare_dram_parameter("w", [NLAYERS, MM, P, KK * P], DT, isOutput=False)
    bias_p = nc.declare_dram_parameter("bias", [NLAYERS, P, MM], f32, isOutput=False)
    out_p = nc.declare_dram_parameter("out", [MM, P, BC], f32, isOutput=True)

    add = mybir.AluOpType.add
    amax = mybir.AluOpType.max

    with tile.TileContext(nc) as tc:
        with (
            tc.tile_pool(name="const", bufs=1) as const,
            tc.tile_pool(name="hA", bufs=1) as hA_pool,
            tc.tile_pool(name="hB", bufs=1) as hB_pool,
            tc.tile_pool(name="wts", bufs=WBUFS) as wpool,
            tc.tile_pool(name="xst", bufs=2) as xpool,
            tc.tile_pool(name="tmp", bufs=3) as tpool,
            tc.tile_pool(name="outt", bufs=3) as opool,
            tc.tile_pool(name="sml", bufs=2) as spool,
            tc.tile_pool(name="psum", bufs=4, space="PSUM") as psum_pool,
            tc.tile_pool(name="psum_s", bufs=1, space="PSUM") as psum_s,
        ):
            bias_t = const.tile([P, NLAYERS * MM], f32)
            for l in range(NLAYERS):
                nc.sync.dma_start(out=bias_t[:, l * MM : (l + 1) * MM], in_=bias_p[l])
            if USE_F32R:
                # memset can't write f32r; stage via f32 + DVE copy
                ones_f = const.tile([P, 1], f32)
                nc.any.memset(ones_f[:], 1.0)
                ones_t = const.tile([P, 1], DT)
                nc.vector.tensor_copy(out=ones_t[:], in_=ones_f[:])
                halves_f = const.tile([1, P], f32)
                nc.any.memset(halves_f[:], 0.5)
                halves_t = const.tile([1, P], DT)
                nc.vector.tensor_copy(out=halves_t[:], in_=halves_f[:])
            else:
                ones_t = const.tile([P, 1], DT)
                nc.any.memset(ones_t[:], 1.0)
                halves_t = const.tile([1, P], DT)
                nc.any.memset(halves_t[:], 0.5)

            for s in range(S):
                c0 = s * COLS
                A = [hA_pool.tile([P, COLS], DT, name=f"hA{k}") for k in range(KK)]
                Bb = [hB_pool.tile([P, COLS], DT, name=f"hB{k}") for k in range(KK)]
                for kk in range(KK):
                    nc.sync.dma_start(out=A[kk][:], in_=xt_p[kk][:, c0 : c0 + COLS])

                # sB[pi] = 0.5 * rowsum(x) broadcast over partitions
                sB = []
                for pi in range(PI):
                    csl = slice(pi * NB, (pi + 1) * NB)
                    ps_s = psum_s.tile([1, NB], f32, name="ps_s")
                    for kk in range(KK):
                        nc.tensor.matmul(
                            ps_s[:],
                            ones_t[:],
                            A[kk][:, csl],
                            start=(kk == 0),
                            stop=(kk == KK - 1),
                        )
                    s_sb = spool.tile([1, NB], DT, name="s_sb")
                    nc.vector.tensor_copy(out=s_sb[:], in_=ps_s[:])
                    ps_b = psum_s.tile([P, NB], f32, name="ps_b")
                    nc.tensor.matmul(
                        ps_b[:], halves_t[:], s_sb[:], start=True, stop=True
                    )
                    sBt = spool.tile([P, NB], f32, name=f"sB{pi}")
                    nc.vector.tensor_copy(out=sBt[:], in_=ps_b[:])
                    sB.append(sBt)

                # layer chain A->B->A->B->A; the FM term is added IN PLACE
                # into A (h4half) right after each layer-3 m-tile evicts, so
                # it fully overlaps layer 3 instead of serializing before
                # layer 4 (no WAR against layer-3's reads of Bb).
                srcs = [A, Bb, A, Bb, A]
                dsts = [Bb, A, Bb, A, None]
                for l in range(NLAYERS):
                    src, dst = srcs[l], dsts[l]
                    for m in range(MM):
                        wt = wpool.tile([P, KK * P], DT, name="wt")
                        nc.sync.dma_start(out=wt[:], in_=w_p[l, m])
                        for pi in range(PI):
                            csl = slice(pi * NB, (pi + 1) * NB)
                            ps = psum_pool.tile([P, NB], f32, name="ps")
                            for kk in range(KK):
                                nc.tensor.matmul(
                                    ps[:],
                                    wt[:, kk * P : (kk + 1) * P],
                                    src[kk][:, csl],
                                    start=(kk == 0),
                                    stop=(kk == KK - 1),
                                )
                            bsl = bias_t[:, l * MM + m : l * MM + m + 1]
                            if l < 4:
                                if USE_F32R:
                                    # dst = max(psum + bias, 0) in one DVE op
                                    # (ACT can't produce f32r outputs)
                                    nc.vector.tensor_scalar(
                                        out=dst[m][:, csl],
                                        in0=ps[:],
                                        scalar1=bsl,
                                        scalar2=0.0,
                                        op0=add,
                                        op1=amax,
                                    )
                                else:
                                    # keep DVE free for the FM-term build;
                                    # ScalarE is otherwise idle
                                    nc.scalar.activation(
                                        dst[m][:, csl],
                                        ps[:],
                                        mybir.ActivationFunctionType.Relu,
                                        bias=bsl,
                                    )
                            else:
                                ot = opool.tile([P, NB], f32, name="ot")
                                nc.vector.tensor_scalar_add(
                                    out=ot[:], in0=ps[:], scalar1=bsl
                                )
                                nc.sync.dma_start(
                                    out=out_p[m][:, c0 + pi * NB : c0 + (pi + 1) * NB],
                                    in_=ot[:],
                                )
                    if l == 3:
                        # A[kk] += relu(x^T * 0.5 s)   (h5in build, in place)
                        for kk in range(KK):
                            xst = xpool.tile([P, COLS], DT, name="xst")
                            nc.sync.dma_start(
                                out=xst[:], in_=xt_p[kk][:, c0 : c0 + COLS]
                            )
                            for pi in range(PI):
                                csl = slice(pi * NB, (pi + 1) * NB)
                                tmp = tpool.tile([P, NB], f32, name="tmp")
                                nc.vector.tensor_mul(
                                    out=tmp[:], in0=xst[:, csl], in1=sB[pi][:]
                                )
                                # A = max(tmp, 0) + A in one fused DVE op
                                nc.vector.scalar_tensor_tensor(
                                    out=A[kk][:, csl],
                                    in0=tmp[:],
                                    scalar=0.0,
                                    in1=A[kk][:, csl],
                                    op0=amax,
                                    op1=add,
                                )
    nc.compile()
    return nc


_NC_CACHE = {}


def _get_nc():
    if "nc" not in _NC_CACHE:
        _NC_CACHE["nc"] = _build()
    return _NC_CACHE["nc"]


def _prep_weights(W_dnn, W_out, b_dnn, b_out):
    w_all = np.empty((NLAYERS, MM, P, KK * P), dtype=NPDT)
    for l in range(NLAYERS):
        W = np.asarray(W_dnn[l] if l < L else W_out, dtype=np.float32)  # [E, Din]
        if l == 3:
            W = W * 0.5  # fold the (h+inter)*0.5 into layer 3's output
        # w[l, m, p, kk*P + j] = W[m*P + j, kk*P + p]
        w_all[l] = (
            W.reshape(MM, P, KK, P)
            .transpose(0, 3, 2, 1)
            .reshape(MM, P, KK * P)
            .astype(NPDT)
        )
    b_all = np.empty((NLAYERS, P, MM), dtype=np.float32)
    for l in range(NLAYERS):
        bl = np.asarray(b_dnn[l] if l < L else b_out, dtype=np.float32)
        if l == 3:
            bl = bl * 0.5
        b_all[l] = bl.reshape(MM, P).T
    return w_all, b_all


def kernel(x, W_dnn, b_dnn, W_out, b_out):
    x = np.asarray(x, dtype=np.float32)
    w_all, b_all = _prep_weights(W_dnn, W_out, b_dnn, b_out)
    nc = _get_nc()
    in_maps = []
    for c in range(NCORES):
        xc = x[c * BC : (c + 1) * BC]  # [BC, D]
        xt = np.ascontiguousarray(xc.T).astype(NPDT).reshape(KK, P, BC)
        in_maps.append({"xt": xt, "w": w_all, "bias": b_all})
    res = run_bass_kernel_spmd(nc, in_maps, list(range(NCORES)))
    out = np.empty((B, D), dtype=np.float32)
    for c in range(NCORES):
        out[c * BC : (c + 1) * BC] = res.results[c]["out"].reshape(D, BC).T
    return out



# revision 3
# speedup vs baseline: 1.3673x; 1.3673x over previous
"""DeepFM dense-MLP kernel for 8x Trainium2 NeuronCores (Bass/Tile).

Computation (reference):
    inter = relu(x * x.sum(axis=1, keepdims=True))        # FM pairwise term
    h = x
    for i in 0..3:  h = relu(h @ W_dnn[i].T + b_dnn[i])
    out = ((h + inter) * 0.5) @ W_out.T + b_out

Strategy:
  - Data-parallel: batch B=8192 split across 8 cores (1024 rows each).
  - Feature-major activations on device: h^T [D, B_c] so every GEMM is
    psum[e, b] += W^T.T @ h^T with the weight stationary.
  - Mixed precision: the final output is dominated by the FM term
    (RMS ~45) while the DNN-chain h4 has RMS ~1, so quantization error
    in layers 0..3 is diluted ~50x.  Layers 0..3 therefore run in fp8
    e4m3 with perf_mode=DoubleRow (2 MACs/PE/cycle, 157 TF/s peak);
    the FM term, rowsum, and the final GEMM stay bf16.  Measured on
    CPU: rel err 4.2e-3 vs 3.9e-3 for all-bf16 (gate 2e-2).
  - fp8 scaling: e4m3 min normal is 2^-6, weights have std 0.02, so
    weights are pre-scaled x64 and activations x16 on the way in; the
    PSUM eviction (ScalarE activation: relu(psum*scale + bias)) folds
    the de-scale, bias, relu, and re-quantization into one op.
  - DoubleRow operand layout: lhsT [128, 2, 128], rhs [128, 2, COLS];
    the pair dim j extends the contraction: k = t*256 + j*128 + p.
  - S=2 super-passes over batch columns keep SBUF under budget
    (weights stream twice; DMA stays well under the 358 GB/s roofline).
  - Rowsum s = 0.5*sum_d x[b,d] is precomputed on host (trivial O(B*D)
    pass) and broadcast across partitions on-device via a K=1 matmul.
"""

import sys

import ml_dtypes
import numpy as np

if "/opt/trn_rl_repo" not in sys.path:
    sys.path.insert(0, "/opt/trn_rl_repo")

import concourse.mybir as mybir  # noqa: E402
import concourse.tile as tile  # noqa: E402
from concourse import bacc  # noqa: E402
from concourse.bass_utils import run_bass_kernel_spmd  # noqa: E402

B, D, L = 8192, 4096, 4
NCORES = 8
BC = B // NCORES  # 1024 batch rows per core
P = 128
KK = D // P  # 32 bf16 k-tiles
KK2 = KK // 2  # 16 fp8 pair k-tiles
MM = D // P  # 32 m-tiles
S = 2  # super-passes over batch columns
COLS = BC // S  # 512
NL = 5

WS = 64.0  # fp8 weight pre-scale (2^6)
HS = 16.0  # fp8 activation pre-scale (2^4)

f32 = mybir.dt.float32
bf16 = mybir.dt.bfloat16
f8 = mybir.dt.float8e4
np_bf16 = ml_dtypes.bfloat16
np_f8 = ml_dtypes.float8_e4m3
DR = mybir.MatmulPerfMode.DoubleRow


def _build():
    nc = bacc.Bacc(None, target_bir_lowering=False, debug=False)
    x8_p = nc.declare_dram_parameter("x8", [KK2, P, 2, BC], f8, isOutput=False)
    xb_p = nc.declare_dram_parameter("xb", [KK, P, BC], bf16, isOutput=False)
    w8_p = nc.declare_dram_parameter("w8", [L, MM, P, KK2, 2, P], f8, isOutput=False)
    wb_p = nc.declare_dram_parameter("wb", [MM, P, KK * P], bf16, isOutput=False)
    bias_p = nc.declare_dram_parameter("bias", [NL, P, MM], f32, isOutput=False)
    sh_p = nc.declare_dram_parameter("sh", [1, BC], bf16, isOutput=False)
    out_p = nc.declare_dram_parameter("out", [MM, P, BC], f32, isOutput=True)

    add = mybir.AluOpType.add
    amax = mybir.AluOpType.max
    relu = mybir.ActivationFunctionType.Relu

    with tile.TileContext(nc) as tc:
        with (
            tc.tile_pool(name="const", bufs=1) as const,
            tc.tile_pool(name="x8t", bufs=1) as x8_pool,
            tc.tile_pool(name="hA", bufs=1) as hA_pool,
            tc.tile_pool(name="hB", bufs=1) as hB_pool,
            tc.tile_pool(name="h5", bufs=1) as h5_pool,
            tc.tile_pool(name="w8t", bufs=6) as w8_pool,
            tc.tile_pool(name="wbt", bufs=4) as wb_pool,
            tc.tile_pool(name="xst", bufs=2) as xpool,
            tc.tile_pool(name="tmp", bufs=3) as tpool,
            tc.tile_pool(name="outt", bufs=3) as opool,
            tc.tile_pool(name="sml", bufs=2) as spool,
            tc.tile_pool(name="psum", bufs=4, space="PSUM") as psum_pool,
            tc.tile_pool(name="psum_s", bufs=1, space="PSUM") as psum_s,
        ):
            bias_t = const.tile([P, NL * MM], f32)
            for l in range(NL):
                nc.sync.dma_start(out=bias_t[:, l * MM : (l + 1) * MM], in_=bias_p[l])
            ones1 = const.tile([1, P], bf16)
            nc.any.memset(ones1[:], 1.0)

            for s in range(S):
                c0 = s * COLS
                X8 = [
                    x8_pool.tile([P, 2, COLS], f8, name=f"x8_{t}") for t in range(KK2)
                ]
                HA = [
                    hA_pool.tile([P, 2, COLS], f8, name=f"hA{t}") for t in range(KK2)
                ]
                HB = [
                    hB_pool.tile([P, 2, COLS], f8, name=f"hB{t}") for t in range(KK2)
                ]
                H5 = [h5_pool.tile([P, COLS], bf16, name=f"h5_{m}") for m in range(MM)]
                for t in range(KK2):
                    nc.sync.dma_start(out=X8[t][:], in_=x8_p[t][:, :, c0 : c0 + COLS])

                # broadcast 0.5*rowsum(x) across partitions via K=1 matmul
                s_sb = spool.tile([1, COLS], bf16, name="s_sb")
                nc.sync.dma_start(out=s_sb[:], in_=sh_p[:, c0 : c0 + COLS])
                ps_b = psum_s.tile([P, COLS], f32, name="ps_b")
                nc.tensor.matmul(ps_b[:], ones1[:], s_sb[:], start=True, stop=True)
                sBt = spool.tile([P, COLS], f32, name="sBt")
                nc.vector.tensor_copy(out=sBt[:], in_=ps_b[:])

                # fp8 DoubleRow layer chain: X8 -> HA -> HB -> HA -> H5(bf16)
                srcs = [X8, HA, HB, HA]
                dsts = [HA, HB, HA, None]
                for l in range(L):
                    src, dst = srcs[l], dsts[l]
                    for m in range(MM):
                        wt = w8_pool.tile([P, KK2, 2, P], f8, name="wt")
                        nc.sync.dma_start(out=wt[:], in_=w8_p[l, m])
                        ps = psum_pool.tile([P, COLS], f32, name="ps")
                        for t in range(KK2):
                            nc.tensor.matmul(
                                ps[:],
                                wt[:, t],
                                src[t][:],
                                start=(t == 0),
                                stop=(t == KK2 - 1),
                                perf_mode=DR,
                            )
                        bsl = bias_t[:, l * MM + m : l * MM + m + 1]
                        if l < 3:
                            # h_next*16 = relu(psum/64 + 16*b); bias is
                            # pre-scaled x16 on host, output re-quantizes
                            # to fp8 in the same ScalarE op
                            nc.scalar.activation(
                                dst[m // 2][:, m % 2, :],
                                ps[:],
                                relu,
                                bias=bsl,
                                scale=1.0 / WS,
                            )
                        else:
                            # 0.5*h4 in bf16: relu(psum/2048 + 0.5*b3)
                            nc.scalar.activation(
                                H5[m][:],
                                ps[:],
                                relu,
                                bias=bsl,
                                scale=0.5 / (WS * HS),
                            )

                # FM term, in place: H5[kk] += relu(xb * 0.5*s)
                for kk in range(KK):
                    xst = xpool.tile([P, COLS], bf16, name="xst")
                    nc.sync.dma_start(out=xst[:], in_=xb_p[kk][:, c0 : c0 + COLS])
                    tmp = tpool.tile([P, COLS], f32, name="tmp")
                    nc.vector.tensor_mul(out=tmp[:], in0=xst[:], in1=sBt[:])
                    nc.vector.scalar_tensor_tensor(
                        out=H5[kk][:],
                        in0=tmp[:],
                        scalar=0.0,
                        in1=H5[kk][:],
                        op0=amax,
                        op1=add,
                    )

                # final layer in bf16
                for m in range(MM):
                    wtb = wb_pool.tile([P, KK * P], bf16, name="wtb")
                    nc.sync.dma_start(out=wtb[:], in_=wb_p[m])
                    ps4 = psum_pool.tile([P, COLS], f32, name="ps")
                    for kk in range(KK):
                        nc.tensor.matmul(
                            ps4[:],
                            wtb[:, kk * P : (kk + 1) * P],
                            H5[kk][:],
                            start=(kk == 0),
                            stop=(kk == KK - 1),
                        )
                    ot = opool.tile([P, COLS], f32, name="ot")
                    nc.vector.tensor_scalar_add(
                        out=ot[:], in0=ps4[:], scalar1=bias_t[:, 4 * MM + m : 4 * MM + m + 1]
                    )
                    nc.sync.dma_start(
                        out=out_p[m][:, c0 : c0 + COLS], in_=ot[:]
                    )
    nc.compile()
    return nc


_NC_CACHE = {}


def _get_nc():
    if "nc" not in _NC_CACHE:
        _NC_CACHE["nc"] = _build()
    return _NC_CACHE["nc"]


def _prep_weights(W_dnn, W_out, b_dnn, b_out):
    w8 = np.empty((L, MM, P, KK2, 2, P), dtype=np_f8)
    for l in range(L):
        Wl = np.asarray(W_dnn[l], np.float32) * WS
        # w8[l, m, p, t, j, col] = WS * W[m*128+col, t*256 + j*128 + p]
        w8[l] = (
            np.clip(Wl, -240.0, 240.0)
            .reshape(MM, P, KK2, 2, P)
            .transpose(0, 4, 2, 3, 1)
            .astype(np_f8)
        )
    Wo = np.asarray(W_out, np.float32)
    wb = (
        Wo.reshape(MM, P, KK, P).transpose(0, 3, 2, 1).reshape(MM, P, KK * P)
    ).astype(np_bf16)
    b_all = np.empty((NL, P, MM), dtype=np.float32)
    for l in range(NL):
        if l < 3:
            bl = np.asarray(b_dnn[l], np.float32) * HS
        elif l == 3:
            bl = np.asarray(b_dnn[3], np.float32) * 0.5
        else:
            bl = np.asarray(b_out, np.float32)
        b_all[l] = bl.reshape(MM, P).T
    return w8, wb, b_all


def _prep_core_inputs(x, w8, wb, b_all):
    in_maps = []
    for c in range(NCORES):
        xc = x[c * BC : (c + 1) * BC]  # [BC, D]
        xcT = np.ascontiguousarray(xc.T)  # [D, BC]
        x8 = (
            np.clip(xcT * HS, -240.0, 240.0)
            .reshape(KK2, 2, P, BC)
            .transpose(0, 2, 1, 3)
            .astype(np_f8)
        )
        xb = xcT.astype(np_bf16).reshape(KK, P, BC)
        sh = (0.5 * xc.sum(axis=1, dtype=np.float32)).astype(np_bf16).reshape(1, BC)
        in_maps.append(
            {"x8": np.ascontiguousarray(x8), "xb": xb, "w8": w8, "wb": wb,
             "bias": b_all, "sh": sh}
        )
    return in_maps


def kernel(x, W_dnn, b_dnn, W_out, b_out):
    x = np.asarray(x, dtype=np.float32)
    w8, wb, b_all = _prep_weights(W_dnn, W_out, b_dnn, b_out)
    nc = _get_nc()
    in_maps = _prep_core_inputs(x, w8, wb, b_all)
    res = run_bass_kernel_spmd(nc, in_maps, list(range(NCORES)))
    out = np.empty((B, D), dtype=np.float32)
    for c in range(NCORES):
        out[c * BC : (c + 1) * BC] = res.results[c]["out"].reshape(D, BC).T
    return out


# revision 5
# speedup vs baseline: 1.6705x; 1.2218x over previous
"""DeepFM dense-MLP kernel for 8x Trainium2 NeuronCores (Bass/Tile).

Computation (reference):
    inter = relu(x * x.sum(axis=1, keepdims=True))        # FM pairwise term
    h = x
    for i in 0..3:  h = relu(h @ W_dnn[i].T + b_dnn[i])
    out = ((h + inter) * 0.5) @ W_out.T + b_out

Strategy:
  - Data-parallel: batch B=8192 split across 8 cores (1024 rows each).
  - Feature-major activations on device: h^T [D, B_c] so every GEMM is
    psum[e, b] += W^T.T @ h^T with the weight stationary.
  - Mixed precision: the final output is dominated by the FM term
    (RMS ~45) while the DNN-chain h4 has RMS ~1, so quantization error
    in layers 0..3 is diluted ~50x.  Layers 0..3 therefore run in fp8
    e4m3 with perf_mode=DoubleRow (2 MACs/PE/cycle); the FM term,
    rowsum, and the final GEMM stay bf16.  Measured rel err 3.5e-3 vs
    3.7e-3 for the all-bf16 version (gate 2e-2).
  - fp8 scaling: e4m3 min normal is 2^-6, weights have std 0.02, so
    weights are pre-scaled x64 and activations x16 on the way in; the
    PSUM eviction (ScalarE activation: relu(psum*scale + bias)) folds
    the de-scale, bias, relu, and re-quantization into one op.
  - DoubleRow operand layout: lhsT [128, 2, 128], rhs [128, 2, COLS];
    the pair dim j extends the contraction: k = t*256 + j*128 + p.
  - S=2 super-passes over batch columns keep SBUF under budget
    (weights stream twice; DMA stays well under the HBM roofline).
  - All weight strips (fp8 + bf16, both super-passes) form one global
    DMA stream issued DEPTH m-tiles ahead of consumption, interleaved
    into the compute loops, so no layer/super-pass boundary ever waits
    on a just-in-time weight transfer.
  - The FM term is built into H5 early (during layer 0, on the
    otherwise-idle DVE): H5 = relu(x * 0.5*s); layer 3's eviction then
    adds 0.5*h4 into it.  Keeps the L3->L4 boundary gapless.
  - Rowsum s = 0.5*sum_d x[b,d] is precomputed on host (trivial) and
    broadcast across partitions on-device via a K=1 matmul.
"""

import sys

import ml_dtypes
import numpy as np

if "/opt/trn_rl_repo" not in sys.path:
    sys.path.insert(0, "/opt/trn_rl_repo")

import concourse.mybir as mybir  # noqa: E402
import concourse.tile as tile  # noqa: E402
from concourse import bacc  # noqa: E402
from concourse.bass_utils import run_bass_kernel_spmd  # noqa: E402

B, D, L = 8192, 4096, 4
NCORES = 8
BC = B // NCORES  # 1024 batch rows per core
P = 128
KK = D // P  # 32 bf16 k-tiles
KK2 = KK // 2  # 16 fp8 pair k-tiles
MM = D // P  # 32 m-tiles
S = 2  # super-passes over batch columns
COLS = BC // S  # 512
NL = 5
DEPTH = 6  # weight-strip DMA lookahead (m-tiles)

WS = 64.0  # fp8 weight pre-scale (2^6)
HS = 16.0  # fp8 activation pre-scale (2^4)

f32 = mybir.dt.float32
bf16 = mybir.dt.bfloat16
f8 = mybir.dt.float8e4
np_bf16 = ml_dtypes.bfloat16
np_f8 = ml_dtypes.float8_e4m3
DR = mybir.MatmulPerfMode.DoubleRow


def _build():
    nc = bacc.Bacc(None, target_bir_lowering=False, debug=False)
    x8_p = nc.declare_dram_parameter("x8", [S, KK2, P, 2, COLS], f8, isOutput=False)
    xb_p = nc.declare_dram_parameter("xb", [KK, P, BC], bf16, isOutput=False)
    w8_p = nc.declare_dram_parameter("w8", [L, MM, P, KK2, 2, P], f8, isOutput=False)
    wb_p = nc.declare_dram_parameter("wb", [MM, P, KK * P], bf16, isOutput=False)
    bias_p = nc.declare_dram_parameter("bias", [NL, P, MM], f32, isOutput=False)
    sh_p = nc.declare_dram_parameter("sh", [1, BC], bf16, isOutput=False)
    out_p = nc.declare_dram_parameter("out", [MM, P, BC], f32, isOutput=True)

    amax = mybir.AluOpType.max
    relu = mybir.ActivationFunctionType.Relu

    with tile.TileContext(nc) as tc:
        with (
            tc.tile_pool(name="const", bufs=1) as const,
            tc.tile_pool(name="x8t", bufs=1) as x8_pool,
            tc.tile_pool(name="hA", bufs=1) as hA_pool,
            tc.tile_pool(name="hB", bufs=1) as hB_pool,
            tc.tile_pool(name="h5", bufs=1) as h5_pool,
            tc.tile_pool(name="w8t", bufs=DEPTH + 2) as w8_pool,
            tc.tile_pool(name="wbt", bufs=DEPTH + 1) as wb_pool,
            tc.tile_pool(name="xst", bufs=4) as xpool,
            tc.tile_pool(name="tmp", bufs=3) as tpool,
            tc.tile_pool(name="outt", bufs=3) as opool,
            tc.tile_pool(name="sml", bufs=2) as spool,
            tc.tile_pool(name="psum", bufs=6, space="PSUM") as psum_pool,
            tc.tile_pool(name="psum_s", bufs=1, space="PSUM") as psum_s,
        ):
            bias_t = const.tile([P, NL * MM], f32)
            for l in range(NL):
                nc.sync.dma_start(out=bias_t[:, l * MM : (l + 1) * MM], in_=bias_p[l])
            ones1 = const.tile([1, P], bf16)
            nc.any.memset(ones1[:], 1.0)

            # global weight stream, in consumption order, both super-passes
            specs = []
            for s in range(S):
                for l in range(L):
                    specs.extend(("f8", l, m) for m in range(MM))
                specs.extend(("bf", None, m) for m in range(MM))
            tiles = {}
            cur = {"issued": 0, "consumed": 0}

            def issue_weight():
                i = cur["issued"]
                if i >= len(specs):
                    return
                cur["issued"] = i + 1
                kind, l, m = specs[i]
                if kind == "f8":
                    wtile = w8_pool.tile([P, KK2, 2, P], f8, name="wt")
                    nc.sync.dma_start(out=wtile[:], in_=w8_p[l, m])
                else:
                    wtile = wb_pool.tile([P, KK * P], bf16, name="wtb")
                    nc.sync.dma_start(out=wtile[:], in_=wb_p[m])
                tiles[i] = wtile

            def consume_weight():
                i = cur["consumed"]
                cur["consumed"] = i + 1
                issue_weight()
                return tiles.pop(i)

            # x8 tiles: one aliased set per super-pass (bufs=1 pool)
            X8 = [
                [x8_pool.tile([P, 2, COLS], f8, name=f"x8_{t}") for t in range(KK2)]
                for s in range(S)
            ]

            issue_weight()
            issue_weight()
            for t in range(KK2):
                nc.sync.dma_start(out=X8[0][t][:], in_=x8_p[0, t])
            for _ in range(DEPTH - 2):
                issue_weight()

            for s in range(S):
                c0 = s * COLS
                HA = [
                    hA_pool.tile([P, 2, COLS], f8, name=f"hA{t}") for t in range(KK2)
                ]
                HB = [
                    hB_pool.tile([P, 2, COLS], f8, name=f"hB{t}") for t in range(KK2)
                ]
                H5 = [h5_pool.tile([P, COLS], bf16, name=f"h5_{m}") for m in range(MM)]

                # broadcast 0.5*rowsum(x) across partitions via K=1 matmul
                s_sb = spool.tile([1, COLS], bf16, name="s_sb")
                nc.sync.dma_start(out=s_sb[:], in_=sh_p[:, c0 : c0 + COLS])
                ps_b = psum_s.tile([P, COLS], f32, name="ps_b")
                nc.tensor.matmul(ps_b[:], ones1[:], s_sb[:], start=True, stop=True)
                sBt = spool.tile([P, COLS], f32, name="sBt")
                nc.vector.tensor_copy(out=sBt[:], in_=ps_b[:])

                # fp8 DoubleRow layer chain: X8 -> HA -> HB -> HA -> (+)H5
                srcs = [X8[s], HA, HB, HA]
                dsts = [HA, HB, HA, None]
                for l in range(L):
                    src, dst = srcs[l], dsts[l]
                    for m in range(MM):
                        wt = consume_weight()
                        ps = psum_pool.tile([P, COLS], f32, name="ps")
                        for t in range(KK2):
                            nc.tensor.matmul(
                                ps[:],
                                wt[:, t],
                                src[t][:],
                                start=(t == 0),
                                stop=(t == KK2 - 1),
                                perf_mode=DR,
                            )
                        bsl = bias_t[:, l * MM + m : l * MM + m + 1]
                        if l < 3:
                            # h_next*16 = relu(psum/64 + 16*b); bias is
                            # pre-scaled x16 on host, output re-quantizes
                            # to fp8 in the same ScalarE op
                            nc.scalar.activation(
                                dst[m // 2][:, m % 2, :],
                                ps[:],
                                relu,
                                bias=bsl,
                                scale=1.0 / WS,
                            )
                        else:
                            # 0.5*h4 = relu(psum/2048 + 0.5*b3), then add
                            # into the FM term built during layer 0
                            t2 = tpool.tile([P, COLS], f32, name="t2")
                            nc.scalar.activation(
                                t2[:], ps[:], relu, bias=bsl, scale=0.5 / (WS * HS)
                            )
                            nc.vector.tensor_add(
                                out=H5[m][:], in0=H5[m][:], in1=t2[:]
                            )
                        if l == 0:
                            # FM term on the idle DVE: H5[m] = relu(xb*0.5s)
                            xst = xpool.tile([P, COLS], bf16, name="xst")
                            nc.sync.dma_start(
                                out=xst[:], in_=xb_p[m][:, c0 : c0 + COLS]
                            )
                            tmp = tpool.tile([P, COLS], f32, name="tmp")
                            nc.vector.tensor_mul(out=tmp[:], in0=xst[:], in1=sBt[:])
                            nc.vector.tensor_scalar_max(
                                out=H5[m][:], in0=tmp[:], scalar1=0.0
                            )

                # final layer in bf16
                for m in range(MM):
                    wtb = consume_weight()
                    ps4 = psum_pool.tile([P, COLS], f32, name="ps")
                    for kk in range(KK):
                        nc.tensor.matmul(
                            ps4[:],
                            wtb[:, kk * P : (kk + 1) * P],
                            H5[kk][:],
                            start=(kk == 0),
                            stop=(kk == KK - 1),
                        )
                    ot = opool.tile([P, COLS], f32, name="ot")
                    nc.vector.tensor_scalar_add(
                        out=ot[:],
                        in0=ps4[:],
                        scalar1=bias_t[:, 4 * MM + m : 4 * MM + m + 1],
                    )
                    nc.sync.dma_start(out=out_p[m][:, c0 : c0 + COLS], in_=ot[:])
                    # prefetch next super-pass's x8 during this one's L4
                    if s + 1 < S and 8 <= m < 8 + KK2:
                        t = m - 8
                        nc.sync.dma_start(
                            out=X8[s + 1][t][:], in_=x8_p[s + 1, t]
                        )
    nc.compile()
    return nc


_NC_CACHE = {}


def _get_nc():
    if "nc" not in _NC_CACHE:
        _NC_CACHE["nc"] = _build()
    return _NC_CACHE["nc"]


def _prep_weights(W_dnn, W_out, b_dnn, b_out):
    w8 = np.empty((L, MM, P, KK2, 2, P), dtype=np_f8)
    for l in range(L):
        Wl = np.asarray(W_dnn[l], np.float32) * WS
        # w8[l, m, p, t, j, col] = WS * W[m*128+col, t*256 + j*128 + p]
        w8[l] = (
            np.clip(Wl, -240.0, 240.0)
            .reshape(MM, P, KK2, 2, P)
            .transpose(0, 4, 2, 3, 1)
            .astype(np_f8)
        )
    Wo = np.asarray(W_out, np.float32)
    wb = (
        Wo.reshape(MM, P, KK, P).transpose(0, 3, 2, 1).reshape(MM, P, KK * P)
    ).astype(np_bf16)
    b_all = np.empty((NL, P, MM), dtype=np.float32)
    for l in range(NL):
        if l < 3:
            bl = np.asarray(b_dnn[l], np.float32) * HS
        elif l == 3:
            bl = np.asarray(b_dnn[3], np.float32) * 0.5
        else:
            bl = np.asarray(b_out, np.float32)
        b_all[l] = bl.reshape(MM, P).T
    return w8, wb, b_all


def _prep_core_inputs(x, w8, wb, b_all):
    in_maps = []
    for c in range(NCORES):
        xc = x[c * BC : (c + 1) * BC]  # [BC, D]
        xcT = np.ascontiguousarray(xc.T)  # [D, BC]
        # x8[s, t, p, j, cols] = HS * x[c0+col, t*256 + j*128 + p]
        x8 = (
            np.clip(xcT * HS, -240.0, 240.0)
            .reshape(KK2, 2, P, S, COLS)
            .transpose(3, 0, 2, 1, 4)
            .astype(np_f8)
        )
        xb = xcT.astype(np_bf16).reshape(KK, P, BC)
        sh = (0.5 * xc.sum(axis=1, dtype=np.float32)).astype(np_bf16).reshape(1, BC)
        in_maps.append(
            {"x8": np.ascontiguousarray(x8), "xb": xb, "w8": w8, "wb": wb,
             "bias": b_all, "sh": sh}
        )
    return in_maps


def kernel(x, W_dnn, b_dnn, W_out, b_out):
    x = np.asarray(x, dtype=np.float32)
    w8, wb, b_all = _prep_weights(W_dnn, W_out, b_dnn, b_out)
    nc = _get_nc()
    in_maps = _prep_core_inputs(x, w8, wb, b_all)
    res = run_bass_kernel_spmd(nc, in_maps, list(range(NCORES)))
    out = np.empty((B, D), dtype=np.float32)
    for c in range(NCORES):
        out[c * BC : (c + 1) * BC] = res.results[c]["out"].reshape(D, BC).T
    return out
